# revision 5
# baseline (speedup 1.0000x reference)
"""Trainium2 Bass kernel for the ADI diffusion layer — band-stencil version.

Math: each ADI step applies three tridiagonal solves (x, y, x) per channel.
The tridiagonal matrices are diagonally dominant with off-diagonal ratio
~5e-4 (coeff = smooth(alpha)*dt/dx^2, dt=1e-3), so each solve operator
T^-1 is numerically *banded*: entries decay ~5e-4 per off-diagonal.
Truncating T^-1 to its tridiagonal band gives a 3-point stencil per sweep
(rel error 2.5e-5 end-to-end vs the fp64 reference; gate is 2e-2).

Reductions on top:
  * adjacent half-step x-sweeps at step boundaries share one operator and
    merge into the band of T^-2: 30 sweeps -> 21;
  * each stencil diagonal is absorbed into a per-element running scale
    (host precompute), so a sweep is out = src + Wm.s(src) + Wp.s(src):
    4 DVE ops per channel (2 broadcast-mults + 2 adds), all at the DVE's
    fast streaming rate - no feedback-limited scans anywhere.

Device dataflow: sweeps alternate x,y,x,...,x. The state flips orientation
at every sweep boundary via PE 128x128 transposes (12 tiles) into PSUM;
the stencil reads PSUM and writes the next SBUF state. The DVE is the only
busy engine (~2.8us/sweep); PE transposes hide underneath it; there are no
scalar-engine copies.  Weights (2 x 384 cols/sweep) stream from HBM one
chunk ahead.  Pure batch data-parallel across 8 cores (B/8 = 4 planes).
"""
import numpy as np

import concourse.bass as bass
from concourse import mybir
from concourse.bass_utils import run_bass_kernel_spmd

# ---- problem constants (hardcoded per contract) ----
B, C, S = 32, 3, 128
NCORES = 8
BL = B // NCORES            # 4 batch planes per core
DT, DX, DY = 0.001, 1.0, 1.0
NUM_STEPS = 10
EPS = 1e-6
NSW = 2 * NUM_STEPS + 1     # 21 sweeps: x, (y, x)*10 with merged double-x
W = BL * S                  # 512 packed free cols per channel
CW = C * W                  # 1536
MCOLS = NSW * 2 * C * S     # 16128 weight cols

F32 = mybir.dt.float32
MUL = mybir.AluOpType.mult
ADD = mybir.AluOpType.add
SUB = mybir.AluOpType.subtract


# ---------------- host-side stencil precompute ----------------

def _smooth(c):
    p = np.pad(c, [(0, 0)] * (c.ndim - 1) + [(1, 1)], mode="edge")
    return (p[..., :-2] + p[..., 1:-1] + p[..., 2:]) / 3.0


def _thomas64(a, b, c, d):
    n = d.shape[-1]
    cs = np.empty_like(d)
    ds = np.empty_like(d)
    den = b[..., 0] + EPS
    cs[..., 0] = c[..., 0] / den
    ds[..., 0] = d[..., 0] / den
    for i in range(1, n):
        den = b[..., i] - a[..., i] * cs[..., i - 1] + EPS
        cs[..., i] = c[..., i] / den
        ds[..., i] = (d[..., i] - a[..., i] * ds[..., i - 1]) / den
    x = np.empty_like(d)
    x[..., -1] = ds[..., -1]
    for i in range(n - 2, -1, -1):
        x[..., i] = ds[..., i] - cs[..., i] * x[..., i + 1]
    return x


def _band_of_solve(coef, dt, dx, power):
    """(Wm, W0, Wp) of the solve operator (or its square) along the last
    axis, extracted with 3-comb solves in fp64. coef: (C,S,S)."""
    coeff = _smooth(coef) * dt / (dx ** 2)
    a = -coeff.copy()
    b = 1.0 + 2.0 * coeff
    b[..., 0] = 1.0 + coeff[..., 0]
    b[..., -1] = 1.0 + coeff[..., -1]
    c = -coeff.copy()
    shp = coef.shape
    n = shp[-1]
    Wm = np.zeros(shp)
    W0 = np.zeros(shp)
    Wp = np.zeros(shp)
    idx = np.arange(n)
    for k in range(3):
        comb = np.zeros(n)
        comb[k::3] = 1.0
        X = _thomas64(a, b, c, np.broadcast_to(comb, shp).copy())
        if power == 2:
            X = _thomas64(a, b, c, X)
        sel0 = (idx % 3) == k
        W0[..., sel0] = X[..., sel0]
        selm = ((idx - 1) % 3) == k
        selm[0] = False
        Wm[..., selm] = X[..., selm]
        selp = ((idx + 1) % 3) == k
        selp[-1] = False
        Wp[..., selp] = X[..., selp]
    return Wm, W0, Wp


def _build_packed(alpha_base, beta_base, alpha_tc, beta_tc):
    """Returns (mults (128, MCOLS) f32, aux (128, 512) f32).
    mults per sweep s: cols [768s,768s+384) Wm-block (c-major 128 cols),
    [768s+384,768s+768) Wp-block, in the sweep's own orientation
    (partition = first spatial axis, free = solve axis).
    aux: [ID | SF]."""
    f8 = np.float64
    ab, bb = alpha_base.astype(f8), beta_base.astype(f8)
    atc, btc = alpha_tc.astype(f8), beta_tc.astype(f8)
    clamp = lambda base, tc, t: np.maximum(base + tc * t, EPS)

    sw = [("x", 0.0, DT / 2, 1)]
    for k in range(NUM_STEPS):
        t = k * DT
        sw.append(("y", t + DT / 2, DT, 1))
        sw.append(("x", t + DT, DT / 2, 2 if k < NUM_STEPS - 1 else 1))

    mults = np.zeros((128, MCOLS), dtype=np.float32)
    S_run = np.ones((C, S, S), dtype=f8)     # x-orientation (c, h, w)
    for s, (which, tt, dt_, power) in enumerate(sw):
        if which == "x":
            coef = clamp(ab, atc, tt)
            Sv = S_run
        else:
            coef = np.swapaxes(clamp(bb, btc, tt), -1, -2)
            Sv = np.swapaxes(S_run, -1, -2)
        Wm, W0, Wp = _band_of_solve(coef, dt_, DX if which == "x" else DY,
                                    power)
        Sp = W0 * Sv
        Wmt = np.zeros_like(Wm)
        Wpt = np.zeros_like(Wp)
        Wmt[..., 1:] = Wm[..., 1:] * Sv[..., :-1] / Sp[..., 1:]
        Wpt[..., :-1] = Wp[..., :-1] * Sv[..., 1:] / Sp[..., :-1]
        S_run = Sp if which == "x" else np.swapaxes(Sp, -1, -2)
        mults[:, 768 * s:768 * s + 384] = \
            Wmt.astype(np.float32).transpose(1, 0, 2).reshape(128, 384)
        mults[:, 768 * s + 384:768 * (s + 1)] = \
            Wpt.astype(np.float32).transpose(1, 0, 2).reshape(128, 384)

    aux = np.zeros((128, 512), dtype=np.float32)
    aux[:, 0:128] = np.eye(128, dtype=np.float32)
    aux[:, 128:512] = \
        S_run.astype(np.float32).transpose(1, 0, 2).reshape(128, 384)
    return mults, aux


# ---------------- device program ----------------

def build_program(repeat=1, final_mult=True):
    nc = bass.Bass("TRN2", target_bir_lowering=False, debug=False)

    u_in = nc.dram_tensor("u", [128, CW], F32, kind="ExternalInput")
    m_in = nc.dram_tensor("mults", [128, MCOLS], F32, kind="ExternalInput")
    x_in = nc.dram_tensor("aux", [128, 512], F32, kind="ExternalInput")
    o_out = nc.dram_tensor("out", [128, CW], F32, kind="ExternalOutput")

    def vcnt(rep, s, c):        # v_sem after add2 of (rep, s, c)
        return 3 * (NSW * rep + s) + c + 1

    def bcnt(rep, s, c):        # pe_sem after transpose group (rep, s, c)
        return 3 * (20 * rep + s) + c + 1   # boundaries s = 0..19 per rep

    with (
        nc.sbuf_tensor([128, MCOLS], F32) as Mt,
        nc.sbuf_tensor([128, CW], F32) as Vx,
        nc.sbuf_tensor([128, CW], F32) as Vy,
        nc.sbuf_tensor([128, CW], F32) as T1,
        nc.sbuf_tensor([128, CW], F32) as T2,
        nc.sbuf_tensor([128, 512], F32) as AX,
        nc.psum_tensor([128, CW], F32) as Px,
        nc.psum_tensor([128, CW], F32) as Py,
        nc.semaphore() as dma_sem,
        nc.semaphore() as v_sem,
        nc.semaphore() as pe_sem,
        nc.Block() as block,
    ):
        ID = AX[:, 0:128]

        def r3(ap2):
            return ap2.rearrange("p (r n) -> p r n", r=BL)

        def ch(t, c):
            return r3(t[:, W * c:W * (c + 1)])

        def wbc(s, half, c, lo, hi):   # weight cols [lo,hi) bcast over r
            base = 768 * s + 384 * half + 128 * c
            return Mt[:, base + lo:base + hi].unsqueeze(1).broadcast_to(
                [128, BL, hi - lo])

        @block.vector
        def _(vector):
            # one-time zero of the stencil guard columns (n=0 of T1,
            # n=127 of T2) using ID cols (finite) as the source
            vector.wait_ge(dma_sem, 16 * 3)
            g1 = T1[:].rearrange("p (c r n) -> p c r n", c=C, r=BL)[
                :, :, :, 0:1]
            g2 = T2[:].rearrange("p (c r n) -> p c r n", c=C, r=BL)[
                :, :, :, S - 1:S]
            zsrc = AX[:, 0:1].unsqueeze(1).unsqueeze(1).broadcast_to(
                [128, C, BL, 1])
            nc.vector.tensor_tensor(g1, zsrc, zsrc, SUB)
            nc.vector.tensor_tensor(g2, zsrc, zsrc, SUB)

            for rep in range(repeat):
                for s in range(NSW):
                    xs = s % 2 == 0
                    Vl = Vx if xs else Vy
                    for c in range(C):
                        if rep == 0 and c == 0:
                            vector.wait_ge(dma_sem, 16 * (3 + s))
                        first = s == 0
                        if not first:
                            vector.wait_ge(pe_sem, bcnt(rep, s - 1, c))
                            src = ch(Px if xs else Py, c)
                        else:
                            src = ch(Vx, c)   # rep>0: written by same engine
                        vc = ch(Vl, c)
                        t1, t2 = ch(T1, c), ch(T2, c)
                        nc.vector.tensor_tensor(
                            t1[:, :, 1:S], wbc(s, 0, c, 1, S),
                            src[:, :, 0:S - 1], MUL)
                        nc.vector.tensor_tensor(
                            t2[:, :, 0:S - 1], wbc(s, 1, c, 0, S - 1),
                            src[:, :, 1:S], MUL)
                        nc.vector.tensor_tensor(vc, src, t1, ADD)
                        nc.vector.tensor_tensor(vc, vc, t2, ADD
                                                ).then_inc(v_sem, 1)
            if final_mult:
                for c in range(C):
                    sf = AX[:, 128 + 128 * c:128 + 128 * (c + 1)]
                    nc.vector.tensor_tensor(
                        ch(T1, c), ch(Vx, c),
                        sf.unsqueeze(1).broadcast_to([128, BL, S]), MUL,
                    ).then_inc(v_sem, 1)

        @block.tensor
        def _(tensor):
            tensor.wait_ge(dma_sem, 16 * 3)
            for rep in range(repeat):
                for s in range(NSW - 1):        # boundaries 0..19
                    src_t = Vx if s % 2 == 0 else Vy
                    dst_t = Py if s % 2 == 0 else Px
                    for c in range(C):
                        tensor.wait_ge(v_sem, vcnt(rep, s, c))
                        last = None
                        for q in range(BL):
                            o = W * c + 128 * q
                            last = nc.tensor.transpose(
                                dst_t[:, o:o + 128], src_t[:, o:o + 128],
                                ID)
                        last.then_inc(pe_sem, 1)

        @block.sync
        def _(sync):
            sync.dma_start(Vx[:], u_in[:]).then_inc(dma_sem, 16)
            sync.dma_start(AX[:], x_in[:]).then_inc(dma_sem, 16)
            sync.dma_start(Mt[:, 0:768], m_in[:, 0:768]).then_inc(dma_sem, 16)
            sync.wait_ge(dma_sem, 16 * 3)
            for s in range(1, NSW):
                sync.dma_start(
                    Mt[:, 768 * s:768 * (s + 1)],
                    m_in[:, 768 * s:768 * (s + 1)],
                ).then_inc(dma_sem, 16)
                sync.wait_ge(dma_sem, 16 * (3 + s))
            vfin = 3 * NSW * repeat
            for c in range(C):
                sync.wait_ge(v_sem, vfin + c + 1)
                sync.dma_start(
                    o_out[:, W * c:W * (c + 1)], T1[:, W * c:W * (c + 1)]
                ).then_inc(dma_sem, 16)

    return nc


_PROGRAM = None


def _get_program():
    global _PROGRAM
    if _PROGRAM is None:
        _PROGRAM = build_program()
    return _PROGRAM


def pack_u(u_core):
    """(BL,C,S,S) -> (128, C*BL*S) device layout (h, c, b, w)."""
    return np.ascontiguousarray(
        u_core.transpose(2, 1, 0, 3).reshape(128, CW), dtype=np.float32)


def unpack_out(o_core):
    """(128, C*BL*S) -> (BL,C,S,S)."""
    return np.ascontiguousarray(
        o_core.reshape(128, C, BL, S).transpose(2, 1, 0, 3))


def make_in_maps(u, alpha_base, beta_base, alpha_time_coeff, beta_time_coeff):
    mults, aux = _build_packed(alpha_base, beta_base,
                               alpha_time_coeff, beta_time_coeff)
    u = np.ascontiguousarray(u, dtype=np.float32)
    return [
        {"u": pack_u(u[i * BL:(i + 1) * BL]), "mults": mults, "aux": aux}
        for i in range(NCORES)
    ]


def kernel(u, alpha_base, beta_base, alpha_time_coeff, beta_time_coeff,
           **run_kwargs):
    in_maps = make_in_maps(u, alpha_base, beta_base,
                           alpha_time_coeff, beta_time_coeff)
    nc = _get_program()
    res = None
    last_err = None
    for _attempt in range(3):
        try:
            res = run_bass_kernel_spmd(nc, in_maps, list(range(NCORES)),
                                       **run_kwargs)
            break
        except Exception as e:  # transient NRT device wedges; retry
            last_err = e
    if res is None:
        raise last_err
    out = np.concatenate(
        [unpack_out(res.results[i]["out"]) for i in range(NCORES)], axis=0)
    return np.ascontiguousarray(out, dtype=np.float32)
